# revision 40
# baseline (speedup 1.0000x reference)
"""Causal self-attention (B=2, T=2048, C=1024, H=16, D=64) on 8 trn2 NeuronCores.

Sharding: tensor-parallel over heads. Each core owns 2 heads:
  - W_attn columns for its heads (q/k/v slices)  -> per-core qkv width 384
  - W_proj rows for its heads                    -> per-core partial output
Attention is embarrassingly parallel over (B, head). Each core returns a
partial [B*T, C] output (y_local @ W_proj_shard) in fp16; the host sums
the 8 partials in float64 (the row-parallel unshard reduction).

Per-core kernel (matmul operands fp16, all accumulation fp32 in PSUM,
~7e-4 relative error vs the fp32 reference):
  0. Host pre-casts everything to fp16 and pre-transposes x: the kernel
     receives x^T [C, BT] fp16 plus W tiles in their final SBUF layouts,
     so the device does no casting and no x transposes. ALL input DMAs
     are issued up front across three different DGE queues (sync /
     gpsimd / scalar) into a persistent [128, 8, 8, 512] SBUF tile, so
     no engine queue ever makes compute wait mid-kernel. A dummy exp at
     t~0 pre-loads the ScalarE activation table off the critical path.
  1. qkvT [384, 4096] = W^T x^T accumulated over 8 C-tiles into single-bank
     PSUM tiles, evacuated to a combined fp16 qkv^T buffer. The v^T third
     is re-transposed by xbar DMAs into natural V_aug tiles
     [Tk 128, v_h0|1|pad|v_h1|1|pad] (stride 160 keeps both xbar
     destinations 32B-aligned); each head's stationary operand is a
     contiguous 65-column slice whose ones-column computes the softmax
     denominators for free.
  2. Scores TRANSPOSED: S^T [Tk 128, Tq 512] = k^T.T @ q^T (K=64), four
     key-tiles per 4-bank PSUM tile so exp runs 2048 wide on ScalarE,
     fused with the PSUM evacuation (psum f32 -> sbuf fp16). Softmax
     max-subtraction is skipped (logits ~N(0,1) after the 1/8 scale; exp
     cannot overflow). The diagonal key-tile group goes last; its matmul
     columns are trimmed to the causal region (col >= 128*u) and only
     the 128x128 diagonal sub-blocks need a GPSIMD affine_select (the
     stop-carrying final AV matmul is full width, so its whole tile is
     masked).
  3. y^T [65, Tq] accumulated over key tiles (lhsT = V_aug, rhs = P^T);
     row 64 is the denominator. Its reciprocal is taken on the compact
     [1, 512] row BEFORE a DRAM bounce + 64-partition broadcast
     (0-stride SWDGE DMA), so no engine's in-order queue ever blocks on
     the broadcast; the trailing multiply (GPSIMD, two chunks behind)
     is the only consumer.
  4. out partial [Tq 128, 1024] = y^T.T @ W_proj_shard: most tiles are
     interleaved into the attention stream via the feed queue (using
     the stage-1 PSUM banks); the rest run in a dedicated phase with a
     4-buf PSUM pool and full-tile evacuations alternating ScalarE/DVE.

The QK->exp->AV softmax chain through the single 4-bank S-PSUM tile is
the serial floor of attention, so emission keeps the PE busy during the
exp windows with a global rate-limited feed queue (2 units per group):
stage-1 matmul pairs for later chunks and projection tiles, flushed by
deadline before the attention call that needs them. That also keeps the
PE's DVFS p-state at full clock (512-col fp16 matmuls run 213ns at
2.4 GHz but 427ns after any stall drops the ramp).
"""
import sys
import numpy as np

if "/opt/trn_rl_repo" not in sys.path:
    sys.path.insert(0, "/opt/trn_rl_repo")

B, T, C, H = 2, 2048, 1024, 16
D = C // H            # 64
NCORES = 8
HPC = H // NCORES     # 2 heads per core
BT = B * T            # 4096
QKV = 3 * HPC * D     # 384 per-core qkv width
NCH = BT // 512       # 8 T-chunks of 512
KT = T // 128         # 16 key tiles per batch
ICH = T // 512        # 4 query chunks per batch
VS = 160              # stride of V_aug tiles: [v_h0|1|pad|v_h1|1|pad], 32B-aligned xbar dests

_PROG = None


def _split_wide_waits(nc, max_waits=1):
    """walrus in this build accepts a single sync wait per instruction;
    Tile's tail drain aggregates one per engine/DMA lane. Split them."""
    import concourse.mybir as mybir
    ctr = 0
    for f in nc.m.functions:
        for bb in f.blocks:
            out = []
            for inst in bb.instructions:
                si = inst.sync_info
                if si is not None and si.on_wait and len(si.on_wait) > max_waits:
                    waits = list(si.on_wait)
                    chunks = [waits[i:i + max_waits]
                              for i in range(0, len(waits), max_waits)]
                    for ch in chunks[:-1]:
                        nop = mybir.InstDrain(
                            name=f"I-wsplit-{ctr}", ins=[], outs=[])
                        ctr += 1
                        nop.engine = inst.engine
                        nop.sync_info = mybir.SyncInfo(on_wait=ch, on_update=[])
                        out.append(nop)
                    inst.sync_info = mybir.SyncInfo(
                        on_wait=chunks[-1],
                        on_update=list(si.on_update) if si.on_update else [])
                out.append(inst)
            bb.instructions = out


def _build_program():
    import concourse.bass as bass
    import concourse.mybir as mybir
    import concourse.tile as tile

    f32 = mybir.dt.float32
    f16 = mybir.dt.float16
    AF = mybir.ActivationFunctionType
    ALU = mybir.AluOpType

    nc = bass.Bass()
    # x^T, pre-cast + pre-transposed on host
    x_d = nc.declare_dram_parameter("xT", [C, BT], f16, isOutput=False)
    # weights already in the SBUF layouts (host-prepared, fp16)
    wqkv_d = nc.declare_dram_parameter("wqkv", [128, 8 * QKV], f16, isOutput=False)
    wproj_d = nc.declare_dram_parameter("wproj", [HPC * D, C], f16, isOutput=False)
    out_d = nc.declare_dram_parameter("out", [BT, C], f16, isOutput=True)
    rs_d = nc.dram_tensor("rscratch", [4, T], f32)  # sums bounce for bcast

    with tile.TileContext(nc) as tc:
        with tc.tile_pool(name="const", bufs=1) as const, \
             tc.tile_pool(name="persist", bufs=1) as persist:
            w_h = const.tile([128, 8 * QKV], f16)
            wp_h = const.tile([128, C], f16)

            # combined qkv^T buffer: q at 0, k at BT, v^T at 2*BT
            qkv_sb = persist.tile([128, 3 * BT], f16)
            v_sb = persist.tile([128, 32 * VS], f16)   # V_aug tiles
            y0 = persist.tile([128, T], f16)           # y^T per b
            y1 = persist.tile([128, T], f16)
            ys = [y0, y1]
            rbc0 = persist.tile([128, T], f32)         # broadcast denominators
            rbc1 = persist.tile([128, T], f32)
            rbcs = [rbc0, rbc1]
            sums_st = persist.tile([1, 4 * T], f32)  # per-pair denominators
            vv = v_sb.rearrange("p (j s) -> p j s", s=VS)

            xt = persist.tile([128, 8, 8, 512], f16)  # all of x^T on-chip
            o_pool = tc.alloc_tile_pool(name="osb", bufs=6)

            with tc.tile_pool(name="wload", bufs=1) as wload, \
                 tc.tile_pool(name="pp", bufs=5) as p_pool, \
                 tc.tile_pool(name="mm_ps", bufs=2, space="PSUM") as mm_ps, \
                 tc.tile_pool(name="s_ps", bufs=1, space="PSUM") as s_ps, \
                 tc.tile_pool(name="y_ps", bufs=2, space="PSUM") as y_ps:
                # all input DMAs issued up front on three different queues,
                # so no engine queue ever makes compute wait mid-kernel:
                # xt[p, ch, cb, t] = xT[cb*128 + p, ch*512 + t]
                def load_chunk(eng, ch):
                    eng.dma_start(
                        out=xt[:, ch],
                        in_=x_d[:, ch * 512:(ch + 1) * 512].rearrange(
                            "(cb p) t -> p cb t", p=128),
                    )
                load_chunk(nc.sync, 0)
                nc.gpsimd.dma_start(out=w_h[:, :], in_=wqkv_d[:, :])
                nc.gpsimd.dma_start(out=wp_h[:, :], in_=wproj_d[:, :])
                for ch in range(1, 4):
                    load_chunk(nc.gpsimd, ch)
                for ch in range(4, 8):
                    load_chunk(nc.scalar, ch)
                ones32 = wload.tile([128, 32], f16)
                nc.vector.memset(ones32[:, :], 1.0)
                nc.vector.tensor_copy(vv[:, :, 64], ones32[:, :])
                nc.vector.tensor_copy(vv[:, :, 144], ones32[:, :])
                # prime the ScalarE activation table (EXP) at t~0 so the
                # 1.3us ACT_TABLE_LOAD is not on the first softmax's path
                warm = wload.tile([128, 1], f16)
                nc.scalar.activation(warm[:, :], ones32[:, 0:1],
                                     AF.Exp, scale=0.125)

                def stage1_units(ch):
                    """Stage-1 for one chunk as a list of thunks (2 matmuls
                    each, plus evac / V-transpose tails) so the QKV work can
                    be interleaved into the attention groups to fill the
                    PE's exp-wait gaps."""
                    accs = {}
                    units = []

                    def mk_mm(m, pair):
                        def th():
                            if pair == 0:
                                accs[m] = mm_ps.tile([128, 512], f32,
                                                     tag="mm",
                                                     name=f"acc{ch}_{m}")
                            acc = accs[m]
                            for cb in (2 * pair, 2 * pair + 1):
                                nc.tensor.matmul(
                                    acc[:, :],
                                    w_h[:, cb * QKV + m * 128:
                                        cb * QKV + (m + 1) * 128],
                                    xt[:, ch, cb, :],
                                    start=(cb == 0), stop=(cb == 7),
                                )
                            if pair == 3:
                                nc.vector.tensor_copy(
                                    qkv_sb[:, m * BT + ch * 512:
                                           m * BT + (ch + 1) * 512],
                                    accs.pop(m)[:, :])
                        return th

                    for m in range(3):
                        for pair in range(4):
                            units.append(mk_mm(m, pair))

                    def vt():
                        # natural V_aug tiles from the vT third via xbar
                        for h in range(HPC):
                            nc.sync.dma_start(
                                out=vv[:, ch * 4:(ch + 1) * 4,
                                       h * 80:h * 80 + 64],
                                in_=qkv_sb[h * 64:(h + 1) * 64,
                                           2 * BT + ch * 512:
                                           2 * BT + (ch + 1) * 512],
                                transpose=True)
                    units.append(vt)
                    return units

                def emit_stage1(ch):
                    for th in stage1_units(ch):
                        th()

                def emit_attention(b, i, feed, cap=2):
                    # `feed`: (deadline_chunk, thunk) deque of stage-1/proj
                    # work, interleaved between attention groups (at most
                    # `cap` per group) so the PE always has exp-independent
                    # work during the QK->exp->AV chain.
                    qs = b * T + i * 512
                    pt_h = {}
                    yps = [y_ps.tile([65, 512], f32, tag="y",
                                     name=f"yps{b}_{i}_{h}")
                           for h in range(HPC)]

                    def emit_qk(g, h):
                        st = s_ps.tile([128, 4, 512], f32, tag="s",
                                       name=f"st{b}_{i}_{g}_{h}")
                        diag = (g == i)
                        for u in range(4):
                            j = 4 * g + u
                            cs = u * 128 if diag else 0
                            nc.tensor.matmul(
                                st[:, u, cs:512],
                                qkv_sb[h * 64:(h + 1) * 64,
                                       BT + b * T + j * 128:
                                       BT + b * T + (j + 1) * 128],
                                qkv_sb[h * 64:(h + 1) * 64,
                                       qs + cs:qs + 512],
                                start=True, stop=True,
                            )
                        pt = p_pool.tile([128, 4, 512], f16, tag="p",
                                         name=f"pt{b}_{i}_{g}_{h}")
                        nc.scalar.activation(
                            pt.rearrange("p u t -> p (u t)"),
                            st.rearrange("p u t -> p (u t)"),
                            AF.Exp, scale=0.125)
                        if diag:
                            # per 128x128 diagonal sub-block: keep col >= p.
                            # u==3 carries the accumulation stop full width,
                            # so zero everything left of its diagonal too.
                            for u in range(3):
                                nc.gpsimd.affine_select(
                                    out=pt[:, u, u * 128:(u + 1) * 128],
                                    in_=pt[:, u, u * 128:(u + 1) * 128],
                                    compare_op=ALU.is_ge, fill=0.0,
                                    base=0, channel_multiplier=-1,
                                    pattern=[[1, 128]],
                                )
                            nc.gpsimd.affine_select(
                                out=pt[:, 3, :], in_=pt[:, 3, :],
                                compare_op=ALU.is_ge, fill=0.0,
                                base=-384, channel_multiplier=-1,
                                pattern=[[1, 512]],
                            )
                        pt_h[(g, h)] = pt

                    def emit_av(g, h):
                        pt = pt_h.pop((g, h))
                        diag = (g == i)
                        for u in range(4):
                            j = 4 * g + u
                            jg = b * KT + j
                            start = (g == 0 and u == 0)
                            stop = (diag and u == 3)
                            cs = u * 128 if (diag and u < 3) else 0
                            nc.tensor.matmul(
                                yps[h][0:65, cs:512],
                                v_sb[:, jg * VS + h * 80:
                                     jg * VS + h * 80 + 65],
                                pt[:, u, cs:512],
                                start=start, stop=stop,
                            )

                    # groups ascending, diagonal last (its V_aug transpose is
                    # kicked off from inside this attention call via `feed`)
                    seq = [(g, h) for g in range(i + 1) for h in range(HPC)]
                    W = 1
                    for n, (g, h) in enumerate(seq):
                        emit_qk(g, h)
                        if n >= W:
                            emit_av(*seq[n - W])
                        # interleave pending filler work (stage-1 for later
                        # chunks / projection tiles), rate-limited so it
                        # lasts through the whole exp chain
                        k = cap
                        while feed and k > 0:
                            feed.popleft()[1]()
                            k -= 1
                    for g, h in seq[max(0, len(seq) - W):]:
                        emit_av(g, h)

                    for h in range(HPC):
                        p = b * HPC + h
                        sl = slice(i * 512, (i + 1) * 512)
                        nc.vector.tensor_copy(
                            ys[b][h * 64:(h + 1) * 64, sl],
                            yps[h][0:64, :])
                        ssl = slice(p * T + i * 512, p * T + (i + 1) * 512)
                        nc.vector.tensor_copy(
                            sums_st[0:1, ssl], yps[h][64:65, :])
                        # reciprocal BEFORE the DRAM bounce (same DVE cost on
                        # [1,512] — free-size bound), so no engine ever waits
                        # on the broadcast except the trailing multiply
                        nc.vector.reciprocal_approx_fast(
                            sums_st[0:1, ssl], sums_st[0:1, ssl])
                        nc.gpsimd.dma_start(out=rs_d[p:p + 1, sl],
                                            in_=sums_st[0:1, ssl])
                        nc.gpsimd.dma_start(
                            out=rbcs[b][h * 64:(h + 1) * 64, sl],
                            in_=rs_d[p:p + 1, sl]
                            .partition_broadcast(64).squeeze(1),
                        )

                def emit_norm_chunk(b, i):
                    # rbcs already holds broadcast reciprocals; just multiply
                    # (on GPSIMD — keeps DVE free for stage-1/attention evacs)
                    sl = slice(i * 512, (i + 1) * 512)
                    nc.gpsimd.tensor_mul(ys[b][:, sl], ys[b][:, sl],
                                         rbcs[b][:, sl])

                def emit_proj_mm(b, ts_):
                    # projection tile using the (idle by now) stage-1 PSUM
                    # banks — interleaved into the last attention call
                    o_sb = o_pool.tile([128, C], f16, tag="o",
                                       name=f"osbf{b}_{ts_}")
                    for n in range(2):
                        op = mm_ps.tile([128, 512], f32, tag="mm",
                                        name=f"opf{b}_{ts_}_{n}")
                        nc.tensor.matmul(
                            op[:, :],
                            ys[b][:, ts_ * 128:(ts_ + 1) * 128],
                            wp_h[:, n * 512:(n + 1) * 512],
                            start=True, stop=True,
                        )
                        if n == 0:
                            nc.scalar.copy(o_sb[:, 0:512], op[:, :])
                        else:
                            nc.vector.tensor_copy(o_sb[:, 512:1024], op[:, :])
                    nc.sync.dma_start(
                        out=out_d[b * T + ts_ * 128:
                                  b * T + (ts_ + 1) * 128, :],
                        in_=o_sb[:, :])

                # stage-1 for later chunks is interleaved INTO attention via
                # a global rate-limited feed queue, so the PE always has
                # exp-independent matmuls during the softmax chain; the
                # queue is flushed of chunk-ch work before attention(ch)
                # needs it. The tail of the queue is the first projection
                # tiles. Per-chunk normalization trails attention by two
                # chunks so the DRAM-bounce broadcast never blocks any
                # engine's in-order queue.
                from collections import deque
                emit_stage1(0)
                feed_q = deque()
                for ch in range(8):
                    b, i = divmod(ch, 4)
                    if ch + 1 < NCH:
                        feed_q.extend((ch + 1, th)
                                      for th in stage1_units(ch + 1))
                    if ch == 6:
                        feed_q.extend((99, (lambda t=t: emit_proj_mm(0, t)))
                                      for t in range(4))
                    if ch == 7:
                        feed_q.extend((99, (lambda t=t: emit_proj_mm(0, t)))
                                      for t in range(4, 16))
                        feed_q.extend((99, (lambda t=t: emit_proj_mm(1, t)))
                                      for t in range(4))
                    # deadline: everything chunk <= ch must be emitted now
                    while feed_q and feed_q[0][0] <= ch:
                        feed_q.popleft()[1]()
                    emit_attention(b, i, feed_q)
                    if ch >= 2:
                        emit_norm_chunk(*divmod(ch - 2, 4))
                while feed_q:
                    feed_q.popleft()[1]()
                emit_norm_chunk(1, 2)
                emit_norm_chunk(1, 3)

            # projection phase: attention PSUM pools are closed above, so
            # all 8 banks are available (4 bufs x 2 banks); each 2-bank
            # group is evacuated by ONE wide copy, alternating ScalarE/DVE
            # (GPSIMD cannot read PSUM) so the PE is never blocked on a
            # single evacuation engine.
            with tc.tile_pool(name="pj_ps", bufs=4, space="PSUM") as pj_ps:
                def emit_proj_tile(b, ts_):
                    o_sb = o_pool.tile([128, C], f16, tag="o",
                                       name=f"osb{b}_{ts_}")
                    op = pj_ps.tile([128, 2, 512], f32, tag="pj",
                                    name=f"ops{b}_{ts_}")
                    for n in range(2):
                        nc.tensor.matmul(
                            op[:, n, :],
                            ys[b][:, ts_ * 128:(ts_ + 1) * 128],
                            wp_h[:, n * 512:(n + 1) * 512],
                            start=True, stop=True,
                        )
                    # alternate full-tile evacuations: each engine handles
                    # every other tile, so the two run fully in parallel
                    opf = op.rearrange("p n t -> p (n t)")
                    if ts_ % 2:
                        nc.scalar.copy(o_sb[:, :], opf[:, :])
                    else:
                        nc.vector.tensor_copy(o_sb[:, :], opf[:, :])
                    nc.sync.dma_start(
                        out=out_d[b * T + ts_ * 128:
                                  b * T + (ts_ + 1) * 128, :],
                        in_=o_sb[:, :])

                for ts_ in range(4, KT):
                    emit_proj_tile(1, ts_)
            o_pool.release()

    # populate .instr bytes for extended-inst InstISA subclasses (the
    # custom-DVE reciprocal) — raw Bass doesn't run this pass itself.
    mybir.codegen_inst_isa_subclasses(nc)
    _split_wide_waits(nc)
    return nc


def _get_program():
    global _PROG
    if _PROG is None:
        _PROG = _build_program()
    return _PROG


def _make_in_maps(x, W_attn, W_proj):
    x = np.asarray(x, dtype=np.float32).reshape(BT, C)
    xT = np.ascontiguousarray(x.T.astype(np.float16))  # [C, BT] fp16
    W_attn = np.asarray(W_attn, dtype=np.float32)
    W_proj = np.asarray(W_proj, dtype=np.float32)
    in_maps = []
    for c in range(NCORES):
        lo = c * HPC * D
        hi = lo + HPC * D
        wqkv = np.concatenate(
            [W_attn[:, lo:hi], W_attn[:, C + lo:C + hi],
             W_attn[:, 2 * C + lo:2 * C + hi]], axis=1)  # [C, QKV]
        # SBUF layout [p, cb, m] = wqkv[cb*128 + p, m], flattened
        wqkv = np.ascontiguousarray(
            wqkv.reshape(8, 128, QKV).transpose(1, 0, 2)
            .reshape(128, 8 * QKV).astype(np.float16))
        wproj = np.ascontiguousarray(W_proj[lo:hi, :].astype(np.float16))
        in_maps.append({"xT": xT, "wqkv": wqkv, "wproj": wproj})
    return in_maps


def kernel(x, W_attn, W_proj):
    from concourse.bass_utils import run_bass_kernel_spmd

    in_maps = _make_in_maps(x, W_attn, W_proj)
    nc = _get_program()
    res = run_bass_kernel_spmd(nc, in_maps, list(range(NCORES)))
    out = res.results[0]["out"].astype(np.float64)
    for c in range(1, NCORES):
        out += res.results[c]["out"]
    return out.astype(np.float32).reshape(B, T, C)


# revision 43
# speedup vs baseline: 1.0253x; 1.0253x over previous
"""Causal self-attention (B=2, T=2048, C=1024, H=16, D=64) on 8 trn2 NeuronCores.

Sharding: tensor-parallel over heads. Each core owns 2 heads:
  - W_attn columns for its heads (q/k/v slices)  -> per-core qkv width 384
  - W_proj rows for its heads                    -> per-core partial output
Attention is embarrassingly parallel over (B, head). Each core returns a
partial [B*T, C] output (y_local @ W_proj_shard) in fp16; the host sums
the 8 partials in float64 (the row-parallel unshard reduction).

Per-core kernel (matmul operands fp16, all accumulation fp32 in PSUM,
~7e-4 relative error vs the fp32 reference):
  0. Host pre-casts everything to fp16 and pre-transposes x: the kernel
     receives x^T [C, BT] fp16 plus W tiles in their final SBUF layouts,
     so the device does no casting and no x transposes. ALL input DMAs
     are issued up front across three different DGE queues (sync /
     gpsimd / scalar) into a persistent [128, 8, 8, 512] SBUF tile, so
     no engine queue ever makes compute wait mid-kernel. A dummy exp at
     t~0 pre-loads the ScalarE activation table off the critical path.
  1. qkvT [384, 4096] = W^T x^T accumulated over 8 C-tiles into single-bank
     PSUM tiles, evacuated to a combined fp16 qkv^T buffer. The v^T third
     is re-transposed by xbar DMAs into natural V_aug tiles
     [Tk 128, v_h0|1|pad|v_h1|1|pad] (stride 160 keeps both xbar
     destinations 32B-aligned); each head's stationary operand is a
     contiguous 65-column slice whose ones-column computes the softmax
     denominators for free.
  2. Scores TRANSPOSED: S^T [Tk 128, Tq 512] = k^T.T @ q^T (K=64), four
     key-tiles per 4-bank PSUM tile so exp runs 2048 wide on ScalarE,
     fused with the PSUM evacuation (psum f32 -> sbuf fp16). Softmax
     max-subtraction is skipped (logits ~N(0,1) after the 1/8 scale; exp
     cannot overflow). The diagonal key-tile group goes last; its matmul
     columns are trimmed to the causal region (col >= 128*u) and only
     the 128x128 diagonal sub-blocks need a GPSIMD affine_select (the
     stop-carrying final AV matmul is full width, so its whole tile is
     masked).
  3. y^T [65, Tq] accumulated over key tiles (lhsT = V_aug, rhs = P^T);
     row 64 is the denominator. Its reciprocal is taken on the compact
     [1, 512] row BEFORE a DRAM bounce + 64-partition broadcast
     (0-stride SWDGE DMA), so no engine's in-order queue ever blocks on
     the broadcast; the trailing multiply (GPSIMD, two chunks behind)
     is the only consumer.
  4. out partial [Tq 128, 1024] = y^T.T @ W_proj_shard: most tiles are
     interleaved into the attention stream via the feed queue (using
     the stage-1 PSUM banks); the rest run in a dedicated phase with a
     4-buf PSUM pool and full-tile evacuations alternating ScalarE/DVE.

The QK->exp->AV softmax chain through the single 4-bank S-PSUM tile is
the serial floor of attention, so emission keeps the PE busy during the
exp windows with a global rate-limited feed queue (2 units per group):
stage-1 matmul pairs for later chunks and projection tiles, flushed by
deadline before the attention call that needs them. That also keeps the
PE's DVFS p-state at full clock (512-col fp16 matmuls run 213ns at
2.4 GHz but 427ns after any stall drops the ramp).
"""
import sys
import numpy as np

if "/opt/trn_rl_repo" not in sys.path:
    sys.path.insert(0, "/opt/trn_rl_repo")

B, T, C, H = 2, 2048, 1024, 16
D = C // H            # 64
NCORES = 8
HPC = H // NCORES     # 2 heads per core
BT = B * T            # 4096
QKV = 3 * HPC * D     # 384 per-core qkv width
NCH = BT // 512       # 8 T-chunks of 512
KT = T // 128         # 16 key tiles per batch
ICH = T // 512        # 4 query chunks per batch
VS = 160              # stride of V_aug tiles: [v_h0|1|pad|v_h1|1|pad], 32B-aligned xbar dests

_PROG = None


def _split_wide_waits(nc, max_waits=1):
    """walrus in this build accepts a single sync wait per instruction;
    Tile's tail drain aggregates one per engine/DMA lane. Split them."""
    import concourse.mybir as mybir
    ctr = 0
    for f in nc.m.functions:
        for bb in f.blocks:
            out = []
            for inst in bb.instructions:
                si = inst.sync_info
                if si is not None and si.on_wait and len(si.on_wait) > max_waits:
                    waits = list(si.on_wait)
                    chunks = [waits[i:i + max_waits]
                              for i in range(0, len(waits), max_waits)]
                    for ch in chunks[:-1]:
                        nop = mybir.InstDrain(
                            name=f"I-wsplit-{ctr}", ins=[], outs=[])
                        ctr += 1
                        nop.engine = inst.engine
                        nop.sync_info = mybir.SyncInfo(on_wait=ch, on_update=[])
                        out.append(nop)
                    inst.sync_info = mybir.SyncInfo(
                        on_wait=chunks[-1],
                        on_update=list(si.on_update) if si.on_update else [])
                out.append(inst)
            bb.instructions = out


def _build_program():
    import concourse.bass as bass
    import concourse.mybir as mybir
    import concourse.tile as tile

    f32 = mybir.dt.float32
    f16 = mybir.dt.float16
    AF = mybir.ActivationFunctionType
    ALU = mybir.AluOpType

    nc = bass.Bass()
    # x^T, pre-cast + pre-transposed on host
    x_d = nc.declare_dram_parameter("xT", [C, BT], f16, isOutput=False)
    # weights already in the SBUF layouts (host-prepared, fp16)
    wqkv_d = nc.declare_dram_parameter("wqkv", [128, 8 * QKV], f16, isOutput=False)
    wproj_d = nc.declare_dram_parameter("wproj", [HPC * D, C], f16, isOutput=False)
    out_d = nc.declare_dram_parameter("out", [BT, C], f16, isOutput=True)
    rs_d = nc.dram_tensor("rscratch", [4, T], f32)  # sums bounce for bcast

    with tile.TileContext(nc) as tc:
        with tc.tile_pool(name="const", bufs=1) as const, \
             tc.tile_pool(name="persist", bufs=1) as persist:
            w_h = const.tile([128, 8 * QKV], f16)
            wp_h = const.tile([128, C], f16)

            # combined qkv^T buffer: q at 0, k at BT, v^T at 2*BT
            qkv_sb = persist.tile([128, 3 * BT], f16)
            v_sb = persist.tile([128, 32 * VS], f16)   # V_aug tiles
            y0 = persist.tile([128, T], f16)           # y^T per b
            y1 = persist.tile([128, T], f16)
            ys = [y0, y1]
            rbc0 = persist.tile([128, T], f32)         # broadcast denominators
            rbc1 = persist.tile([128, T], f32)
            rbcs = [rbc0, rbc1]
            sums_st = persist.tile([1, 4 * T], f32)  # per-pair denominators
            vv = v_sb.rearrange("p (j s) -> p j s", s=VS)

            xt = persist.tile([128, 8, 8, 512], f16)  # all of x^T on-chip
            o_pool = tc.alloc_tile_pool(name="osb", bufs=8)

            with tc.tile_pool(name="wload", bufs=1) as wload, \
                 tc.tile_pool(name="pp", bufs=5) as p_pool, \
                 tc.tile_pool(name="mm_ps", bufs=2, space="PSUM") as mm_ps, \
                 tc.tile_pool(name="s_ps", bufs=1, space="PSUM") as s_ps, \
                 tc.tile_pool(name="y_ps", bufs=2, space="PSUM") as y_ps:
                # all input DMAs issued up front on three different queues,
                # so no engine queue ever makes compute wait mid-kernel:
                # xt[p, ch, cb, t] = xT[cb*128 + p, ch*512 + t]
                def load_chunk(eng, ch):
                    eng.dma_start(
                        out=xt[:, ch],
                        in_=x_d[:, ch * 512:(ch + 1) * 512].rearrange(
                            "(cb p) t -> p cb t", p=128),
                    )
                load_chunk(nc.sync, 0)
                nc.gpsimd.dma_start(out=w_h[:, :], in_=wqkv_d[:, :])
                nc.gpsimd.dma_start(out=wp_h[:, :], in_=wproj_d[:, :])
                for ch in range(1, 4):
                    load_chunk(nc.gpsimd, ch)
                for ch in range(4, 8):
                    load_chunk(nc.scalar, ch)
                ones32 = wload.tile([128, 32], f16)
                nc.vector.memset(ones32[:, :], 1.0)
                nc.vector.tensor_copy(vv[:, :, 64], ones32[:, :])
                nc.vector.tensor_copy(vv[:, :, 144], ones32[:, :])
                # prime the ScalarE activation table (EXP) at t~0 so the
                # 1.3us ACT_TABLE_LOAD is not on the first softmax's path
                warm = wload.tile([128, 1], f16)
                nc.scalar.activation(warm[:, :], ones32[:, 0:1],
                                     AF.Exp, scale=0.125)

                def stage1_units(ch):
                    """Stage-1 for one chunk as a list of thunks (2 matmuls
                    each, plus evac / V-transpose tails) so the QKV work can
                    be interleaved into the attention groups to fill the
                    PE's exp-wait gaps."""
                    accs = {}
                    units = []

                    def mk_mm(m, pair):
                        def th():
                            if pair == 0:
                                accs[m] = mm_ps.tile([128, 512], f32,
                                                     tag="mm",
                                                     name=f"acc{ch}_{m}")
                            acc = accs[m]
                            for cb in (2 * pair, 2 * pair + 1):
                                nc.tensor.matmul(
                                    acc[:, :],
                                    w_h[:, cb * QKV + m * 128:
                                        cb * QKV + (m + 1) * 128],
                                    xt[:, ch, cb, :],
                                    start=(cb == 0), stop=(cb == 7),
                                )
                            if pair == 3:
                                nc.vector.tensor_copy(
                                    qkv_sb[:, m * BT + ch * 512:
                                           m * BT + (ch + 1) * 512],
                                    accs.pop(m)[:, :])
                        return th

                    for m in range(3):
                        for pair in range(4):
                            units.append(mk_mm(m, pair))

                    def vt():
                        # natural V_aug tiles from the vT third via xbar
                        for h in range(HPC):
                            nc.sync.dma_start(
                                out=vv[:, ch * 4:(ch + 1) * 4,
                                       h * 80:h * 80 + 64],
                                in_=qkv_sb[h * 64:(h + 1) * 64,
                                           2 * BT + ch * 512:
                                           2 * BT + (ch + 1) * 512],
                                transpose=True)
                    units.append(vt)
                    return units

                def emit_stage1(ch):
                    for th in stage1_units(ch):
                        th()

                def emit_attention(b, i, feed, cap=2):
                    # `feed`: (deadline_chunk, thunk) deque of stage-1/proj
                    # work, interleaved between attention groups (at most
                    # `cap` per group) so the PE always has exp-independent
                    # work during the QK->exp->AV chain.
                    qs = b * T + i * 512
                    pt_h = {}
                    yps = [y_ps.tile([65, 512], f32, tag="y",
                                     name=f"yps{b}_{i}_{h}")
                           for h in range(HPC)]

                    def emit_qk(g, h):
                        st = s_ps.tile([128, 4, 512], f32, tag="s",
                                       name=f"st{b}_{i}_{g}_{h}")
                        diag = (g == i)
                        for u in range(4):
                            j = 4 * g + u
                            cs = u * 128 if diag else 0
                            nc.tensor.matmul(
                                st[:, u, cs:512],
                                qkv_sb[h * 64:(h + 1) * 64,
                                       BT + b * T + j * 128:
                                       BT + b * T + (j + 1) * 128],
                                qkv_sb[h * 64:(h + 1) * 64,
                                       qs + cs:qs + 512],
                                start=True, stop=True,
                            )
                        pt = p_pool.tile([128, 4, 512], f16, tag="p",
                                         name=f"pt{b}_{i}_{g}_{h}")
                        nc.scalar.activation(
                            pt.rearrange("p u t -> p (u t)"),
                            st.rearrange("p u t -> p (u t)"),
                            AF.Exp, scale=0.125)
                        if diag:
                            # per 128x128 diagonal sub-block: keep col >= p.
                            # u==3 carries the accumulation stop full width,
                            # so zero everything left of its diagonal too.
                            for u in range(3):
                                nc.gpsimd.affine_select(
                                    out=pt[:, u, u * 128:(u + 1) * 128],
                                    in_=pt[:, u, u * 128:(u + 1) * 128],
                                    compare_op=ALU.is_ge, fill=0.0,
                                    base=0, channel_multiplier=-1,
                                    pattern=[[1, 128]],
                                )
                            nc.gpsimd.affine_select(
                                out=pt[:, 3, :], in_=pt[:, 3, :],
                                compare_op=ALU.is_ge, fill=0.0,
                                base=-384, channel_multiplier=-1,
                                pattern=[[1, 512]],
                            )
                        pt_h[(g, h)] = pt

                    def emit_av(g, h):
                        pt = pt_h.pop((g, h))
                        diag = (g == i)
                        for u in range(4):
                            j = 4 * g + u
                            jg = b * KT + j
                            start = (g == 0 and u == 0)
                            stop = (diag and u == 3)
                            cs = u * 128 if (diag and u < 3) else 0
                            nc.tensor.matmul(
                                yps[h][0:65, cs:512],
                                v_sb[:, jg * VS + h * 80:
                                     jg * VS + h * 80 + 65],
                                pt[:, u, cs:512],
                                start=start, stop=stop,
                            )

                    # groups ascending, diagonal last (its V_aug transpose is
                    # kicked off from inside this attention call via `feed`)
                    seq = [(g, h) for g in range(i + 1) for h in range(HPC)]
                    W = 1
                    for n, (g, h) in enumerate(seq):
                        emit_qk(g, h)
                        if n >= W:
                            emit_av(*seq[n - W])
                        # interleave pending filler work (stage-1 for later
                        # chunks / projection tiles), rate-limited so it
                        # lasts through the whole exp chain
                        k = cap
                        while feed and k > 0:
                            feed.popleft()[1]()
                            k -= 1
                    for g, h in seq[max(0, len(seq) - W):]:
                        emit_av(g, h)

                    for h in range(HPC):
                        p = b * HPC + h
                        sl = slice(i * 512, (i + 1) * 512)
                        nc.vector.tensor_copy(
                            ys[b][h * 64:(h + 1) * 64, sl],
                            yps[h][0:64, :])
                        ssl = slice(p * T + i * 512, p * T + (i + 1) * 512)
                        nc.vector.tensor_copy(
                            sums_st[0:1, ssl], yps[h][64:65, :])
                        # reciprocal BEFORE the DRAM bounce (same DVE cost on
                        # [1,512] — free-size bound), so no engine ever waits
                        # on the broadcast except the trailing multiply
                        nc.vector.reciprocal_approx_fast(
                            sums_st[0:1, ssl], sums_st[0:1, ssl])
                        nc.gpsimd.dma_start(out=rs_d[p:p + 1, sl],
                                            in_=sums_st[0:1, ssl])
                        nc.gpsimd.dma_start(
                            out=rbcs[b][h * 64:(h + 1) * 64, sl],
                            in_=rs_d[p:p + 1, sl]
                            .partition_broadcast(64).squeeze(1),
                        )

                def emit_norm_chunk(b, i):
                    # rbcs already holds broadcast reciprocals; just multiply
                    # (on GPSIMD — keeps DVE free for stage-1/attention evacs)
                    sl = slice(i * 512, (i + 1) * 512)
                    nc.gpsimd.tensor_mul(ys[b][:, sl], ys[b][:, sl],
                                         rbcs[b][:, sl])

                def emit_proj_mm(b, ts_):
                    # projection tile using the (idle by now) stage-1 PSUM
                    # banks — interleaved into the last attention call
                    o_sb = o_pool.tile([128, C], f16, tag="o",
                                       name=f"osbf{b}_{ts_}")
                    for n in range(2):
                        op = mm_ps.tile([128, 512], f32, tag="mm",
                                        name=f"opf{b}_{ts_}_{n}")
                        nc.tensor.matmul(
                            op[:, :],
                            ys[b][:, ts_ * 128:(ts_ + 1) * 128],
                            wp_h[:, n * 512:(n + 1) * 512],
                            start=True, stop=True,
                        )
                        if n == 0:
                            nc.scalar.copy(o_sb[:, 0:512], op[:, :])
                        else:
                            nc.vector.tensor_copy(o_sb[:, 512:1024], op[:, :])
                    nc.sync.dma_start(
                        out=out_d[b * T + ts_ * 128:
                                  b * T + (ts_ + 1) * 128, :],
                        in_=o_sb[:, :])

                # stage-1 for later chunks is interleaved INTO attention via
                # a global rate-limited feed queue, so the PE always has
                # exp-independent matmuls during the softmax chain; the
                # queue is flushed of chunk-ch work before attention(ch)
                # needs it. The tail of the queue is the first projection
                # tiles. Per-chunk normalization trails attention by two
                # chunks so the DRAM-bounce broadcast never blocks any
                # engine's in-order queue.
                from collections import deque
                emit_stage1(0)
                feed_q = deque()
                for ch in range(8):
                    b, i = divmod(ch, 4)
                    if ch + 1 < NCH:
                        feed_q.extend((ch + 1, th)
                                      for th in stage1_units(ch + 1))
                    if ch == 6:
                        feed_q.extend((99, (lambda t=t: emit_proj_mm(0, t)))
                                      for t in range(4))
                    if ch == 7:
                        feed_q.extend((99, (lambda t=t: emit_proj_mm(0, t)))
                                      for t in range(4, 16))
                        feed_q.extend((99, (lambda t=t: emit_proj_mm(1, t)))
                                      for t in range(4))
                    # deadline: everything chunk <= ch must be emitted now
                    while feed_q and feed_q[0][0] <= ch:
                        feed_q.popleft()[1]()
                    emit_attention(b, i, feed_q)
                    if ch >= 2:
                        emit_norm_chunk(*divmod(ch - 2, 4))
                while feed_q:
                    feed_q.popleft()[1]()
                emit_norm_chunk(1, 2)
                emit_norm_chunk(1, 3)

            # projection phase: attention PSUM pools are closed above, so
            # all 8 banks are available (4 bufs x 2 banks); each 2-bank
            # group is evacuated by ONE wide copy, alternating ScalarE/DVE
            # (GPSIMD cannot read PSUM) so the PE is never blocked on a
            # single evacuation engine.
            with tc.tile_pool(name="pj_ps", bufs=4, space="PSUM") as pj_ps:
                def emit_proj_tile(b, ts_):
                    o_sb = o_pool.tile([128, C], f16, tag="o",
                                       name=f"osb{b}_{ts_}")
                    op = pj_ps.tile([128, 2, 512], f32, tag="pj",
                                    name=f"ops{b}_{ts_}")
                    for n in range(2):
                        nc.tensor.matmul(
                            op[:, n, :],
                            ys[b][:, ts_ * 128:(ts_ + 1) * 128],
                            wp_h[:, n * 512:(n + 1) * 512],
                            start=True, stop=True,
                        )
                    # alternate full-tile evacuations: each engine handles
                    # every other tile, so the two run fully in parallel
                    opf = op.rearrange("p n t -> p (n t)")
                    if ts_ % 2:
                        nc.scalar.copy(o_sb[:, :], opf[:, :])
                    else:
                        nc.vector.tensor_copy(o_sb[:, :], opf[:, :])
                    nc.sync.dma_start(
                        out=out_d[b * T + ts_ * 128:
                                  b * T + (ts_ + 1) * 128, :],
                        in_=o_sb[:, :])

                for ts_ in range(4, KT):
                    emit_proj_tile(1, ts_)
            o_pool.release()

    # populate .instr bytes for extended-inst InstISA subclasses (the
    # custom-DVE reciprocal) — raw Bass doesn't run this pass itself.
    mybir.codegen_inst_isa_subclasses(nc)
    _split_wide_waits(nc)
    return nc


def _get_program():
    global _PROG
    if _PROG is None:
        _PROG = _build_program()
    return _PROG


def _make_in_maps(x, W_attn, W_proj):
    x = np.asarray(x, dtype=np.float32).reshape(BT, C)
    xT = np.ascontiguousarray(x.T.astype(np.float16))  # [C, BT] fp16
    W_attn = np.asarray(W_attn, dtype=np.float32)
    W_proj = np.asarray(W_proj, dtype=np.float32)
    in_maps = []
    for c in range(NCORES):
        lo = c * HPC * D
        hi = lo + HPC * D
        wqkv = np.concatenate(
            [W_attn[:, lo:hi], W_attn[:, C + lo:C + hi],
             W_attn[:, 2 * C + lo:2 * C + hi]], axis=1)  # [C, QKV]
        # SBUF layout [p, cb, m] = wqkv[cb*128 + p, m], flattened
        wqkv = np.ascontiguousarray(
            wqkv.reshape(8, 128, QKV).transpose(1, 0, 2)
            .reshape(128, 8 * QKV).astype(np.float16))
        wproj = np.ascontiguousarray(W_proj[lo:hi, :].astype(np.float16))
        in_maps.append({"xT": xT, "wqkv": wqkv, "wproj": wproj})
    return in_maps


def kernel(x, W_attn, W_proj):
    from concourse.bass_utils import run_bass_kernel_spmd

    in_maps = _make_in_maps(x, W_attn, W_proj)
    nc = _get_program()
    res = run_bass_kernel_spmd(nc, in_maps, list(range(NCORES)))
    out = res.results[0]["out"].astype(np.float64)
    for c in range(1, NCORES):
        out += res.results[c]["out"]
    return out.astype(np.float32).reshape(B, T, C)
